# revision 44
# baseline (speedup 1.0000x reference)
"""NT-Xent loss kernel for Trainium2, 8 NeuronCores.

Row-sharded similarity matrix, multi-engine exponential pipeline:
  - Each core gets the full feature matrix cyclically rolled by c*1024 rows
    (identical SPMD program; its rows are always rolled-rows [0,1024)).
  - Preamble (per 2048-col group): DMA-in x f32; row norms via
    square+reduce; rnorm = exp(-0.5 ln) on ACT; scale to z in bf16
    (broadcast tensor_tensor); transpose via the SBUF crossbar DMA
    (dma_start_transpose - no PE/DVE time).  Group 0's elementwise work
    runs on DVE (short head chain); later groups' on GPSIMD.
  - Main loop (32 units of [128 rows x 2048 cols]): bf16 matmuls (fp8
    DoubleRow measured SLOWER than bf16 on hw) into PSUM f32; the exp
    work is split between ACT (native Exp + fused accum_out row sums, cols
    [0:ACOLS]) and DVE (Schraudolph exp: one fused tensor_scalar
    f32->int16 whose codes ARE the bf16 bit pattern of exp(x), plus a 2x
    bf16 tensor_reduce for row sums, cols [ACOLS:2048]).
  - The diagonal/positive-pair blocks (units g=0 and g=2, cols m*128..)
    land in the ACT range; those [128,128] code blocks are DMA'd out raw
    and the host extracts diagonals, does ln(rowsum - exp_diag) -
    ln(exp_pos), and means - no device epilogue.
"""

import os

import numpy as np

N = 8192
D = 128
NCORES = 8
RPC = N // NCORES          # rows per core = 1024
G = 4                      # column groups
GCOLS = N // G             # 2048 columns per group
M = RPC // 128             # row tiles per core = 8
ACOLS = 1536               # ACT's exp share per 2048-col tile (mult of 128)
ZSCALE = 1.0               # z kept at unit scale (bf16 matmul)
ESC = 10.0                 # exp scale on raw psum values
SCH_A = 1846.649652        # ESC * 2^7 * log2(e)   (Schraudolph slope)
SCH_B = 16248.7807255      # 127*2^7 mean-centered (Schraudolph offset)

_CACHE = {}
LAST_RESULTS = None


def _patch_act_tables():
    """Force Exp/Ln onto the combined natural_log_exp_and_others table set
    so a single ACT table load covers the whole kernel."""
    if _CACHE.get("act_patched"):
        return
    import functools

    import concourse.bacc as bacc_mod
    import concourse.bass_interp as interp_mod
    import concourse.hw_specs as hw_specs
    import concourse.mybir as mybir

    AF = mybir.ActivationFunctionType
    orig = hw_specs.get_activation_tables

    @functools.cache
    def patched(arch):
        out = {}
        for name, funcs in orig(arch).items():
            if name != "natural_log_exp_and_others":
                funcs = funcs - {AF.Exp, AF.Ln}
            out[name] = funcs
        return out

    hw_specs.get_activation_tables = patched
    bacc_mod.get_activation_tables = patched
    interp_mod.get_activation_tables = patched
    _CACHE["act_patched"] = True


def _patch_ldw_opt():
    """Let walrus dedup consecutive identical LDWEIGHTS; the DoubleRow
    stationary is reused by 4 consecutive matmuls per unit."""
    if _CACHE.get("ldw_patched"):
        return
    import concourse.bass_utils as bu

    orig = bu.run_command

    def run2(argv, **kw):
        argv = [
            "--enable-ldw-opt=true" if a == "--enable-ldw-opt=false" else a
            for a in argv
        ]
        return orig(argv, **kw)

    bu.run_command = run2
    _CACHE["ldw_patched"] = True


def _build():
    import concourse.mybir as mybir
    import concourse.tile as tile
    from concourse import bacc

    _patch_act_tables()

    from bass_rust import add_dep_helper

    f32 = mybir.dt.float32
    bf16 = mybir.dt.bfloat16
    i16 = mybir.dt.int16
    AX = mybir.AxisListType
    OP = mybir.AluOpType
    AF = mybir.ActivationFunctionType

    nc = bacc.Bacc(
        "TRN2",
        target_bir_lowering=False,
        debug=False,
        enable_asserts=False,
        num_devices=NCORES,
    )
    x = nc.dram_tensor("x", [N, D], f32, kind="ExternalInput").ap()
    ident_in = nc.dram_tensor("ident", [128, 128], f32, kind="ExternalInput").ap()
    racc_out = nc.dram_tensor("racc", [128, 2 * G * M + 1], f32, kind="ExternalOutput").ap()
    etblk_out = nc.dram_tensor("etblk", [128, 2 * RPC], i16, kind="ExternalOutput").ap()

    with tile.TileContext(nc) as tc:
        with (
            tc.tile_pool(name="const", bufs=1) as constp,
            tc.tile_pool(name="xin", bufs=3) as xinp,
            tc.tile_pool(name="sq", bufs=2) as sqp,
            tc.tile_pool(name="small", bufs=2) as smallp,
            tc.tile_pool(name="zb", bufs=2) as zbp,
            tc.tile_pool(name="zbT", bufs=2) as zbTp,
            tc.tile_pool(name="zbT0", bufs=1) as zbT0p,
            tc.tile_pool(name="et", bufs=12) as etp,
            tc.tile_pool(name="acc", bufs=1) as accp,
            tc.tile_pool(name="psum", bufs=2, space="PSUM") as psump,
        ):
            # constants
            eps2 = constp.tile([128, 1], f32, tag="eps2")
            nc.vector.memset(eps2[:], 1e-16)
            identf = constp.tile([128, 128], f32, tag="identf")
            nc.sync.dma_start(out=identf[:], in_=ident_in)
            identb = constp.tile([128, 128], bf16, tag="identb")
            nc.vector.tensor_copy(identb[:], identf[:])

            racc = accp.tile([128, 2 * G * M + 1], f32, tag="racc")
            etcol = accp.tile([128, 2 * RPC], i16, tag="etcol")

            xgs = [None] * G
            zbTs = [None] * G
            ets = [None] * M

            def load_part(g, two_rings=False, after=None):
                """DMA group g's 2048 rolled rows of x into SBUF.  after=
                holds the transfers behind an instruction (keeps GPSIMD's
                dependent square off the head's critical window)."""
                xg = xinp.tile([128, GCOLS], f32, tag="xg")
                for q in range(4):
                    src = x[g * GCOLS + q * 512 : g * GCOLS + (q + 1) * 512, :]
                    src = src.rearrange("(s p) d -> p s d", p=128)
                    dst = xg[:, q * 512 : (q + 1) * 512].rearrange(
                        "p (s d) -> p s d", s=4
                    )
                    e = nc.scalar if (two_rings and q % 2 == 0) else nc.sync
                    ins = e.dma_start(out=dst, in_=src)
                    if after is not None:
                        add_dep_helper(ins.ins, after.ins, sync=True,
                                       reason="delay load behind head")
                xgs[g] = xg

            prev_dve = [None]
            chain_ops = []

            def chain(ins, sync=False):
                # pin emission order of the preamble; sync=True enforces
                # across engines (GPSIMD work must not overlap the head's
                # DVE chain - they contend for the shared SBUF port)
                if prev_dve[0] is not None:
                    add_dep_helper(ins.ins, prev_dve[0].ins, sync=sync,
                                   reason="preamble order")
                prev_dve[0] = ins
                chain_ops.append(ins)

            rnos = [None] * G
            sqs = [None] * G
            nsqs = [None] * G
            zbs = [None] * G

            def stage1_sq(g, sync=False):
                """Squares for group g's row norms on GPSIMD.  sync=True
                holds it until the head's DVE chain is done (shared SBUF
                port contention)."""
                xg = xgs[g]
                sq = sqp.tile([128, GCOLS], bf16, tag="sq", name="sq")
                nc.gpsimd.tensor_tensor(sq[:], xg[:], xg[:], op=OP.mult)
                sqs[g] = sq

            def stage1_red(g):
                """Reduce to ||x||^2 (DVE), then rno = exp(-0.5 ln) on ACT."""
                sq = sqs[g]
                nsq = smallp.tile([128, 16], bf16, tag="nsq")
                with nc.allow_low_precision(reason="bf16 norm, 2e-2 tol"):
                    chain(nc.vector.tensor_reduce(
                        nsq[:],
                        sq[:].rearrange("p (s d) -> p s d", s=16),
                        axis=AX.X,
                        op=OP.add,
                    ))
                lnv = smallp.tile([128, 16], f32, tag="lnv")
                nc.scalar.activation(lnv[:], nsq[:], AF.Ln, bias=eps2[:, 0:1])
                rno = smallp.tile([128, 16], f32, tag="rno")
                nc.scalar.activation(rno[:], lnv[:], AF.Exp, scale=-0.5)
                rnos[g] = rno

            def stage2(g, on_dve=False):
                """Scale to z (bf16) and crossbar-transpose."""
                xg, rno = xgs[g], rnos[g]
                zbT = (zbT0p if g == 0 else zbTp).tile(
                    [128, GCOLS], bf16, tag="zbT", name="zbT")
                zb = zbp.tile([128, GCOLS], bf16, tag="zb", name="zb")
                ew = nc.vector if on_dve else nc.gpsimd
                ew.tensor_tensor(
                    zb[:].rearrange("p (s d) -> p s d", s=16),
                    xg[:].rearrange("p (s d) -> p s d", s=16),
                    rno[:].unsqueeze(-1).broadcast_to([128, 16, 128]),
                    op=OP.mult,
                )
                # crossbar transpose: zbT[d, s*128+r] = zb[r, s*128+d]
                nc.sync.dma_start_transpose(
                    out=zbT[:].rearrange("p (s r) -> p s r", s=16),
                    in_=zb[:],
                )
                zbTs[g] = zbT

            def prep_head(g):
                """Group 0: chunked norms + scale on DVE, transposed on the
                (otherwise idle) PE via identity matmuls - much lower
                latency than the crossbar DMA, so the first matmuls start
                ~6us earlier."""
                xg = xgs[g]
                sq = sqp.tile([128, GCOLS], bf16, tag="sq")
                nsq = smallp.tile([128, 16], bf16, tag="nsq")
                lnv = smallp.tile([128, 16], f32, tag="lnv")
                rno = smallp.tile([128, 16], f32, tag="rno")
                zbc = [zbp.tile([128, 512], bf16, tag=f"zbh{c}",
                                name=f"zbh{c}") for c in range(4)]
                zbT = zbT0p.tile([128, GCOLS], bf16, tag="zbT")
                # transpose bounce: a bf16 view of one main psum ring slot
                # (unit(0,0) takes the other slot, so no extra banks and no
                # wait on the first matmuls)
                pth = psump.tile([128, GCOLS], f32, tag="pt", name="pth")
                pthb = pth[:].bitcast(bf16)

                def norm_c(c):
                    cs = slice(c * 512, (c + 1) * 512)
                    ss = slice(c * 4, (c + 1) * 4)
                    chain(nc.vector.tensor_tensor(sq[:, cs], xg[:, cs],
                                                  xg[:, cs], op=OP.mult))
                    with nc.allow_low_precision(reason="bf16 norm, 2e-2 tol"):
                        chain(nc.vector.tensor_reduce(
                            nsq[:, ss],
                            sq[:, cs].rearrange("p (s d) -> p s d", s=4),
                            axis=AX.X,
                            op=OP.add,
                        ))
                    nc.scalar.activation(lnv[:, ss], nsq[:, ss], AF.Ln,
                                         bias=eps2[:, 0:1])
                    nc.scalar.activation(rno[:, ss], lnv[:, ss], AF.Exp,
                                         scale=-0.5)

                def scale_c(c):
                    cs = slice(c * 512, (c + 1) * 512)
                    ss = slice(c * 4, (c + 1) * 4)
                    zb = zbc[c]
                    chain(nc.vector.tensor_tensor(
                        zb[:].rearrange("p (s d) -> p s d", s=4),
                        xg[:, cs].rearrange("p (s d) -> p s d", s=4),
                        rno[:, ss].unsqueeze(-1).broadcast_to([128, 4, 128]),
                        op=OP.mult,
                    ))
                    for j in range(4):
                        nc.tensor.transpose(
                            pthb[:, c * 1024 + j * 128 : c * 1024 + (j + 1) * 128],
                            zb[:, j * 128 : (j + 1) * 128],
                            identb[:],
                        )
                    chain(nc.vector.tensor_copy(
                        zbT[:, cs], pthb[:, c * 1024 : c * 1024 + 512]
                    ))

                zbTs[g] = zbT
                pt0 = psump.tile([128, GCOLS], f32, tag="pt", name="pt0")
                norm_c(0)
                for c in range(1, 4):
                    norm_c(c)
                    scale_c(c - 1)
                    unit_mm(0, 0, pt0, c - 1)
                scale_c(3)
                unit_mm(0, 0, pt0, 3)
                return pt0

            def unit_mm(g, m, pt, k):
                lhs = zbTs[0][:, m * 128 : (m + 1) * 128]
                nc.tensor.matmul(
                    pt[:, k * 512 : (k + 1) * 512],
                    lhs,
                    zbTs[g][:, k * 512 : (k + 1) * 512],
                )

            def unit(g, m, pt=None):
                """One [128 rows x 2048 cols] tile: matmuls + split exp."""
                u = g * M + m
                if pt is None:
                    pt = psump.tile([128, GCOLS], f32, tag="pt")
                    for k in range(4):
                        unit_mm(g, m, pt, k)
                et = etp.tile([128, GCOLS], i16, tag="et")
                # ACT: true exp, fused row-sum accumulation.  The very
                # first unit is split so the exp stream starts right after
                # the first 512-col transpose chunk.
                if u == 0:
                    nc.scalar.activation(
                        et[:, 0:512].bitcast(bf16),
                        pt[:, 0:512],
                        AF.Exp,
                        scale=ESC,
                        accum_out=racc[:, 2 * G * M : 2 * G * M + 1],
                    )
                    nc.scalar.activation(
                        et[:, 512:ACOLS].bitcast(bf16),
                        pt[:, 512:ACOLS],
                        AF.Exp,
                        scale=ESC,
                        accum_out=racc[:, u : u + 1],
                    )
                else:
                    nc.scalar.activation(
                        et[:, 0:ACOLS].bitcast(bf16),
                        pt[:, 0:ACOLS],
                        AF.Exp,
                        scale=ESC,
                        accum_out=racc[:, u : u + 1],
                    )
                if u == G * M - 1:
                    # last unit: ACT takes the whole tile so the kernel tail
                    # is just the accumulator read + the racc DMA
                    nc.scalar.activation(
                        et[:, ACOLS:GCOLS].bitcast(bf16),
                        pt[:, ACOLS:GCOLS],
                        AF.Exp,
                        scale=ESC,
                        accum_out=racc[:, G * M + u : G * M + u + 1],
                    )
                else:
                    # DVE: Schraudolph exp (int16 codes = bf16 bit pattern)
                    nc.vector.tensor_scalar(
                        et[:, ACOLS:GCOLS], pt[:, ACOLS:GCOLS],
                        SCH_A, SCH_B, OP.mult, OP.add,
                    )
                    nc.vector.tensor_reduce(
                        racc[:, G * M + u : G * M + u + 1],
                        et[:, ACOLS:GCOLS].bitcast(bf16),
                        axis=AX.X,
                        op=OP.add,
                    )
                if g == 0 or g == 2:
                    ets[m] = (g, et)

            def collect_etblk():
                """Collect the held diag/pos code blocks on GPSIMD (emitted
                after the next group's stage2 so the GPS preamble never
                blocks) and ship the half to HBM."""
                half = None
                for mm in range(M):
                    gsrc, et = ets[mm]
                    half = 0 if gsrc == 0 else 1
                    nc.gpsimd.tensor_copy(
                        etcol[:, half * RPC + mm * 128 : half * RPC + (mm + 1) * 128],
                        et[:, mm * 128 : (mm + 1) * 128],
                    )
                    ets[mm] = None
                nc.sync.dma_start(
                    out=etblk_out[:, half * RPC : (half + 1) * RPC],
                    in_=etcol[:, half * RPC : (half + 1) * RPC],
                )

            # ---- pipeline ----
            load_part(0, two_rings=True)
            # warm the ACT Ln/Exp table while the input DMAs run
            warm = constp.tile([128, 1], f32, tag="warm")
            nc.vector.memset(warm[:], 1.0)
            nc.scalar.activation(warm[:], warm[:], AF.Ln)
            nc.scalar.activation(warm[:], warm[:], AF.Exp)
            pt0 = prep_head(0)
            load_part(1)
            stage1_sq(1)
            stage1_red(1)
            unit(0, 0, pt=pt0)
            for g in range(G):
                if g + 2 < G:
                    load_part(g + 2)
                for m in range(M):
                    if not (g == 0 and m == 0):
                        unit(g, m)
                    if g + 1 < G and m == 0:
                        stage2(g + 1)
                    if m == 2 and g in (1, 3) and ets[0] is not None:
                        collect_etblk()
                    if g + 2 < G and m == 4:
                        stage1_sq(g + 2)
                    if g + 2 < G and m == 6:
                        stage1_red(g + 2)
            

            nc.sync.dma_start(out=racc_out, in_=racc[:])

    _dedup_ldweights(nc)
    nc.compile()
    return nc


def _dedup_ldweights(nc):
    """Drop InstLdweights whose stationary operand is identical to the
    immediately preceding PE ldweights (the 4 matmuls of a unit share one
    stationary).  Deps/syncs of a dropped ldweights move to the following
    matmul.  Walrus's own --enable-ldw-opt rejects pre-split InstLdweights,
    so this is done at the BIR level."""
    for fn in nc.m.functions:
        for blk in fn.blocks:
            keep = []
            last_sig = None
            drop = []
            for ins in blk.instructions:
                tn = type(ins).__name__
                if getattr(ins, "engine", None) is not None and str(ins.engine) == "EngineType.PE":
                    if tn == "InstLdweights":
                        ap = ins.ins[0]
                        sig = (str(getattr(ap, "memref", None)), ap.offset,
                               str(ap.ap), str(getattr(ap, "dtype", None)))
                        if sig == last_sig and ins.sync_info is None:
                            drop.append(ins)
                            continue
                        last_sig = sig
                    elif tn != "InstMatmult":
                        last_sig = None
                keep.append(ins)
                if drop and tn == "InstMatmult":
                    for dins in drop:
                        ins.merge_dependencies_from(dins)
                    drop = []
            assert not drop
            blk.instructions[:] = keep


def _get_nc():
    if "nc" not in _CACHE:
        _CACHE["nc"] = _build()
    return _CACHE["nc"]


def kernel(stacked_batch: np.ndarray) -> np.ndarray:
    global LAST_RESULTS
    import ml_dtypes
    from concourse.bass_utils import run_bass_kernel_spmd

    nc = _get_nc()
    xf = np.ascontiguousarray(np.asarray(stacked_batch, dtype=np.float32))
    assert xf.shape == (N, D)

    ident = np.eye(128, dtype=np.float32)
    in_maps = [
        {"x": np.ascontiguousarray(np.roll(xf, -c * RPC, axis=0)), "ident": ident}
        for c in range(NCORES)
    ]
    res = run_bass_kernel_spmd(
        nc,
        in_maps,
        core_ids=list(range(NCORES)),
        trace=bool(os.environ.get("BASS_TRACE")),
    )
    LAST_RESULTS = res

    total = 0.0
    idx = np.arange(128)
    for c in range(NCORES):
        racc = np.asarray(res.results[c]["racc"], dtype=np.float64)  # [128, 64]
        etblk = np.asarray(res.results[c]["etblk"])                  # [128, 2048] i16
        vals = etblk.view(np.uint16).view(ml_dtypes.bfloat16).astype(np.float64)
        # rowsum for local row r = m*128 + p: sum ACT + DVE partials over g
        rowsum = np.zeros((128, M))
        for g in range(G):
            for m in range(M):
                u = g * M + m
                rowsum[:, m] += racc[:, u] + racc[:, G * M + u]
        rowsum[:, 0] += racc[:, 2 * G * M]  # split first-unit extra slot
        exp_diag = np.empty((128, M))
        exp_pos = np.empty((128, M))
        for m in range(M):
            exp_diag[:, m] = vals[idx, m * 128 + idx]
            exp_pos[:, m] = vals[idx, RPC + m * 128 + idx]
        loss = np.log(rowsum - exp_diag) - np.log(exp_pos)
        total += float(loss.sum())
    return np.float32(total / N)


# revision 45
# speedup vs baseline: 1.0229x; 1.0229x over previous
"""NT-Xent loss kernel for Trainium2, 8 NeuronCores.

Row-sharded similarity matrix with a multi-engine exponential pipeline:
  - Each core gets the full feature matrix cyclically rolled by c*1024 rows
    (identical SPMD program; its 1024 rows are always rolled-rows [0,1024)).
  - Group 0 preamble (latency-critical head): 512-col chunks on DVE
    (square, bf16 reduce, rnorm = exp(-0.5 ln) on ACT, broadcast scale),
    transposed via identity matmuls on the otherwise-idle PE into a bf16
    bitcast view of a PSUM ring slot; the first unit's matmuls are
    interleaved with the chunks and its ACT exp is split 512/1024 so the
    exp stream starts as early as possible.
  - Groups 1-3 preamble (throughput path): square + scale on GPSIMD
    (group 1's input DMA is held behind the head chain - GPSIMD and DVE
    contend for the shared SBUF port), bf16 reduce on DVE, and the
    transpose on the SBUF crossbar DMA (dma_start_transpose), which costs
    no engine time.
  - Main loop, 32 units of [128 rows x 2048 cols]: bf16 matmuls (fp8
    DoubleRow measured SLOWER than bf16 on this hw) into PSUM f32, one
    LDWEIGHTS per unit (redundant ones deduped at the BIR level since
    walrus --enable-ldw-opt rejects pre-split InstLdweights).  The exp is
    split between ACT (native Exp, fused f32 accum_out row sums, cols
    [0:1536]) and DVE (Schraudolph exp: one fused tensor_scalar
    f32->int16 whose rounded codes ARE the bf16 bit pattern of exp(x),
    mean-centered via the magic offset, plus a bf16 tensor_reduce for the
    row sums, cols [1536:2048]).  The last unit is ACT-only so the tail
    is just the accumulator read and the result DMA.
  - The diagonal (g=0) and positive-pair (g=2) [128,128] code blocks are
    collected by GPSIMD copies and shipped raw; the host extracts the
    diagonals, computes ln(rowsum - exp_diag) - ln(exp_pos) in f64, and
    means across cores - there is no device epilogue.
"""

import os

import numpy as np

N = 8192
D = 128
NCORES = 8
RPC = N // NCORES          # rows per core = 1024
G = 4                      # column groups
GCOLS = N // G             # 2048 columns per group
M = RPC // 128             # row tiles per core = 8
ACOLS = 1536               # ACT's exp share per 2048-col tile (mult of 128)
ZSCALE = 1.0               # z kept at unit scale (bf16 matmul)
ESC = 10.0                 # exp scale on raw psum values
SCH_A = 1846.649652        # ESC * 2^7 * log2(e)   (Schraudolph slope)
SCH_B = 16248.7807255      # 127*2^7 mean-centered (Schraudolph offset)

_CACHE = {}
LAST_RESULTS = None


def _patch_act_tables():
    """Force Exp/Ln onto the combined natural_log_exp_and_others table set
    so a single ACT table load covers the whole kernel."""
    if _CACHE.get("act_patched"):
        return
    import functools

    import concourse.bacc as bacc_mod
    import concourse.bass_interp as interp_mod
    import concourse.hw_specs as hw_specs
    import concourse.mybir as mybir

    AF = mybir.ActivationFunctionType
    orig = hw_specs.get_activation_tables

    @functools.cache
    def patched(arch):
        out = {}
        for name, funcs in orig(arch).items():
            if name != "natural_log_exp_and_others":
                funcs = funcs - {AF.Exp, AF.Ln}
            out[name] = funcs
        return out

    hw_specs.get_activation_tables = patched
    bacc_mod.get_activation_tables = patched
    interp_mod.get_activation_tables = patched
    _CACHE["act_patched"] = True


def _patch_ldw_opt():
    """Let walrus dedup consecutive identical LDWEIGHTS; the DoubleRow
    stationary is reused by 4 consecutive matmuls per unit."""
    if _CACHE.get("ldw_patched"):
        return
    import concourse.bass_utils as bu

    orig = bu.run_command

    def run2(argv, **kw):
        argv = [
            "--enable-ldw-opt=true" if a == "--enable-ldw-opt=false" else a
            for a in argv
        ]
        return orig(argv, **kw)

    bu.run_command = run2
    _CACHE["ldw_patched"] = True


def _build():
    import concourse.mybir as mybir
    import concourse.tile as tile
    from concourse import bacc

    _patch_act_tables()

    from bass_rust import add_dep_helper

    f32 = mybir.dt.float32
    bf16 = mybir.dt.bfloat16
    i16 = mybir.dt.int16
    AX = mybir.AxisListType
    OP = mybir.AluOpType
    AF = mybir.ActivationFunctionType

    nc = bacc.Bacc(
        "TRN2",
        target_bir_lowering=False,
        debug=False,
        enable_asserts=False,
        num_devices=NCORES,
    )
    x = nc.dram_tensor("x", [N, D], f32, kind="ExternalInput").ap()
    ident_in = nc.dram_tensor("ident", [128, 128], f32, kind="ExternalInput").ap()
    racc_out = nc.dram_tensor("racc", [128, 2 * G * M + 1], f32, kind="ExternalOutput").ap()
    etblk_out = nc.dram_tensor("etblk", [128, 2 * RPC], i16, kind="ExternalOutput").ap()

    with tile.TileContext(nc) as tc:
        with (
            tc.tile_pool(name="const", bufs=1) as constp,
            tc.tile_pool(name="xin", bufs=3) as xinp,
            tc.tile_pool(name="sq", bufs=2) as sqp,
            tc.tile_pool(name="small", bufs=2) as smallp,
            tc.tile_pool(name="zb", bufs=2) as zbp,
            tc.tile_pool(name="zbT", bufs=2) as zbTp,
            tc.tile_pool(name="zbT0", bufs=1) as zbT0p,
            tc.tile_pool(name="et", bufs=12) as etp,
            tc.tile_pool(name="acc", bufs=1) as accp,
            tc.tile_pool(name="psum", bufs=2, space="PSUM") as psump,
        ):
            # constants
            eps2 = constp.tile([128, 1], f32, tag="eps2")
            nc.vector.memset(eps2[:], 1e-16)
            identf = constp.tile([128, 128], f32, tag="identf")
            nc.sync.dma_start(out=identf[:], in_=ident_in)
            identb = constp.tile([128, 128], bf16, tag="identb")
            nc.vector.tensor_copy(identb[:], identf[:])

            racc = accp.tile([128, 2 * G * M + 1], f32, tag="racc")
            etcol = accp.tile([128, 2 * RPC], i16, tag="etcol")

            xgs = [None] * G
            zbTs = [None] * G
            ets = [None] * M

            def load_part(g, two_rings=False, after=None):
                """DMA group g's 2048 rolled rows of x into SBUF.  after=
                holds the transfers behind an instruction (keeps GPSIMD's
                dependent square off the head's critical window)."""
                xg = xinp.tile([128, GCOLS], f32, tag="xg")
                for q in range(4):
                    src = x[g * GCOLS + q * 512 : g * GCOLS + (q + 1) * 512, :]
                    src = src.rearrange("(s p) d -> p s d", p=128)
                    dst = xg[:, q * 512 : (q + 1) * 512].rearrange(
                        "p (s d) -> p s d", s=4
                    )
                    e = nc.scalar if (two_rings and q % 2 == 0) else nc.sync
                    ins = e.dma_start(out=dst, in_=src)
                    if after is not None:
                        add_dep_helper(ins.ins, after.ins, sync=True,
                                       reason="delay load behind head")
                xgs[g] = xg

            prev_dve = [None]
            chain_ops = []

            def chain(ins, sync=False):
                # pin emission order of the preamble; sync=True enforces
                # across engines (GPSIMD work must not overlap the head's
                # DVE chain - they contend for the shared SBUF port)
                if prev_dve[0] is not None:
                    add_dep_helper(ins.ins, prev_dve[0].ins, sync=sync,
                                   reason="preamble order")
                prev_dve[0] = ins
                chain_ops.append(ins)

            rnos = [None] * G
            sqs = [None] * G
            nsqs = [None] * G
            zbs = [None] * G

            def stage1_sq(g, sync=False):
                """Squares for group g's row norms on GPSIMD.  sync=True
                holds it until the head's DVE chain is done (shared SBUF
                port contention)."""
                xg = xgs[g]
                sq = sqp.tile([128, GCOLS], bf16, tag="sq", name="sq")
                nc.gpsimd.tensor_tensor(sq[:], xg[:], xg[:], op=OP.mult)
                sqs[g] = sq

            def stage1_red(g):
                """Reduce to ||x||^2 (DVE), then rno = exp(-0.5 ln) on ACT."""
                sq = sqs[g]
                nsq = smallp.tile([128, 16], bf16, tag="nsq")
                with nc.allow_low_precision(reason="bf16 norm, 2e-2 tol"):
                    chain(nc.vector.tensor_reduce(
                        nsq[:],
                        sq[:].rearrange("p (s d) -> p s d", s=16),
                        axis=AX.X,
                        op=OP.add,
                    ))
                lnv = smallp.tile([128, 16], f32, tag="lnv")
                nc.scalar.activation(lnv[:], nsq[:], AF.Ln, bias=eps2[:, 0:1])
                rno = smallp.tile([128, 16], f32, tag="rno")
                nc.scalar.activation(rno[:], lnv[:], AF.Exp, scale=-0.5)
                rnos[g] = rno

            def stage2(g, on_dve=False):
                """Scale to z (bf16) and crossbar-transpose."""
                xg, rno = xgs[g], rnos[g]
                zbT = (zbT0p if g == 0 else zbTp).tile(
                    [128, GCOLS], bf16, tag="zbT", name="zbT")
                zb = zbp.tile([128, GCOLS], bf16, tag="zb", name="zb")
                ew = nc.vector if on_dve else nc.gpsimd
                ew.tensor_tensor(
                    zb[:].rearrange("p (s d) -> p s d", s=16),
                    xg[:].rearrange("p (s d) -> p s d", s=16),
                    rno[:].unsqueeze(-1).broadcast_to([128, 16, 128]),
                    op=OP.mult,
                )
                # crossbar transpose: zbT[d, s*128+r] = zb[r, s*128+d]
                nc.sync.dma_start_transpose(
                    out=zbT[:].rearrange("p (s r) -> p s r", s=16),
                    in_=zb[:],
                )
                zbTs[g] = zbT

            def prep_head(g):
                """Group 0: chunked norms + scale on DVE, transposed on the
                (otherwise idle) PE via identity matmuls - much lower
                latency than the crossbar DMA, so the first matmuls start
                ~6us earlier."""
                xg = xgs[g]
                sq = sqp.tile([128, GCOLS], bf16, tag="sq")
                nsq = smallp.tile([128, 16], bf16, tag="nsq")
                lnv = smallp.tile([128, 16], f32, tag="lnv")
                rno = smallp.tile([128, 16], f32, tag="rno")
                zbc = [zbp.tile([128, 512], bf16, tag=f"zbh{c}",
                                name=f"zbh{c}") for c in range(4)]
                zbT = zbT0p.tile([128, GCOLS], bf16, tag="zbT")
                # transpose bounce: a bf16 view of one main psum ring slot
                # (unit(0,0) takes the other slot, so no extra banks and no
                # wait on the first matmuls)
                pth = psump.tile([128, GCOLS], f32, tag="pt", name="pth")
                pthb = pth[:].bitcast(bf16)

                def norm_c(c):
                    cs = slice(c * 512, (c + 1) * 512)
                    ss = slice(c * 4, (c + 1) * 4)
                    chain(nc.vector.tensor_tensor(sq[:, cs], xg[:, cs],
                                                  xg[:, cs], op=OP.mult))
                    with nc.allow_low_precision(reason="bf16 norm, 2e-2 tol"):
                        chain(nc.vector.tensor_reduce(
                            nsq[:, ss],
                            sq[:, cs].rearrange("p (s d) -> p s d", s=4),
                            axis=AX.X,
                            op=OP.add,
                        ))
                    nc.scalar.activation(lnv[:, ss], nsq[:, ss], AF.Ln,
                                         bias=eps2[:, 0:1])
                    nc.scalar.activation(rno[:, ss], lnv[:, ss], AF.Exp,
                                         scale=-0.5)

                def scale_c(c):
                    cs = slice(c * 512, (c + 1) * 512)
                    ss = slice(c * 4, (c + 1) * 4)
                    zb = zbc[c]
                    chain(nc.vector.tensor_tensor(
                        zb[:].rearrange("p (s d) -> p s d", s=4),
                        xg[:, cs].rearrange("p (s d) -> p s d", s=4),
                        rno[:, ss].unsqueeze(-1).broadcast_to([128, 4, 128]),
                        op=OP.mult,
                    ))
                    for j in range(4):
                        nc.tensor.transpose(
                            pthb[:, c * 1024 + j * 128 : c * 1024 + (j + 1) * 128],
                            zb[:, j * 128 : (j + 1) * 128],
                            identb[:],
                        )
                    chain(nc.vector.tensor_copy(
                        zbT[:, cs], pthb[:, c * 1024 : c * 1024 + 512]
                    ))

                zbTs[g] = zbT
                pt0 = psump.tile([128, GCOLS], f32, tag="pt", name="pt0")
                norm_c(0)
                for c in range(1, 4):
                    norm_c(c)
                    scale_c(c - 1)
                    unit_mm(0, 0, pt0, c - 1)
                scale_c(3)
                unit_mm(0, 0, pt0, 3)
                return pt0

            def unit_mm(g, m, pt, k):
                lhs = zbTs[0][:, m * 128 : (m + 1) * 128]
                nc.tensor.matmul(
                    pt[:, k * 512 : (k + 1) * 512],
                    lhs,
                    zbTs[g][:, k * 512 : (k + 1) * 512],
                )

            def unit(g, m, pt=None):
                """One [128 rows x 2048 cols] tile: matmuls + split exp."""
                u = g * M + m
                if pt is None:
                    pt = psump.tile([128, GCOLS], f32, tag="pt")
                    for k in range(4):
                        unit_mm(g, m, pt, k)
                et = etp.tile([128, GCOLS], i16, tag="et")
                # ACT: true exp, fused row-sum accumulation.  The very
                # first unit is split so the exp stream starts right after
                # the first 512-col transpose chunk.
                if u == 0:
                    nc.scalar.activation(
                        et[:, 0:512].bitcast(bf16),
                        pt[:, 0:512],
                        AF.Exp,
                        scale=ESC,
                        accum_out=racc[:, 2 * G * M : 2 * G * M + 1],
                    )
                    nc.scalar.activation(
                        et[:, 512:ACOLS].bitcast(bf16),
                        pt[:, 512:ACOLS],
                        AF.Exp,
                        scale=ESC,
                        accum_out=racc[:, u : u + 1],
                    )
                else:
                    nc.scalar.activation(
                        et[:, 0:ACOLS].bitcast(bf16),
                        pt[:, 0:ACOLS],
                        AF.Exp,
                        scale=ESC,
                        accum_out=racc[:, u : u + 1],
                    )
                if u == G * M - 1:
                    # last unit: ACT takes the whole tile so the kernel tail
                    # is just the accumulator read + the racc DMA
                    nc.scalar.activation(
                        et[:, ACOLS:GCOLS].bitcast(bf16),
                        pt[:, ACOLS:GCOLS],
                        AF.Exp,
                        scale=ESC,
                        accum_out=racc[:, G * M + u : G * M + u + 1],
                    )
                else:
                    # DVE: Schraudolph exp (int16 codes = bf16 bit pattern)
                    nc.vector.tensor_scalar(
                        et[:, ACOLS:GCOLS], pt[:, ACOLS:GCOLS],
                        SCH_A, SCH_B, OP.mult, OP.add,
                    )
                    nc.vector.tensor_reduce(
                        racc[:, G * M + u : G * M + u + 1],
                        et[:, ACOLS:GCOLS].bitcast(bf16),
                        axis=AX.X,
                        op=OP.add,
                    )
                if g == 0 or g == 2:
                    ets[m] = (g, et)

            def collect_etblk():
                """Collect the held diag/pos code blocks on GPSIMD (emitted
                after the next group's stage2 so the GPS preamble never
                blocks) and ship the half to HBM."""
                half = None
                for mm in range(M):
                    gsrc, et = ets[mm]
                    half = 0 if gsrc == 0 else 1
                    nc.gpsimd.tensor_copy(
                        etcol[:, half * RPC + mm * 128 : half * RPC + (mm + 1) * 128],
                        et[:, mm * 128 : (mm + 1) * 128],
                    )
                    ets[mm] = None
                nc.sync.dma_start(
                    out=etblk_out[:, half * RPC : (half + 1) * RPC],
                    in_=etcol[:, half * RPC : (half + 1) * RPC],
                )

            # ---- pipeline ----
            load_part(0, two_rings=True)
            # warm the ACT Ln/Exp table while the input DMAs run
            warm = constp.tile([128, 1], f32, tag="warm")
            nc.vector.memset(warm[:], 1.0)
            nc.scalar.activation(warm[:], warm[:], AF.Ln)
            nc.scalar.activation(warm[:], warm[:], AF.Exp)
            pt0 = prep_head(0)
            load_part(1, after=chain_ops[10])
            stage1_sq(1)
            stage1_red(1)
            unit(0, 0, pt=pt0)
            for g in range(G):
                if g + 2 < G:
                    load_part(g + 2)
                for m in range(M):
                    if not (g == 0 and m == 0):
                        unit(g, m)
                    if g + 1 < G and m == 0:
                        stage2(g + 1)
                    if m == 2 and g in (1, 3) and ets[0] is not None:
                        collect_etblk()
                    if g + 2 < G and m == 4:
                        stage1_sq(g + 2)
                    if g + 2 < G and m == 6:
                        stage1_red(g + 2)
            

            nc.sync.dma_start(out=racc_out, in_=racc[:])

    _dedup_ldweights(nc)
    nc.compile()
    return nc


def _dedup_ldweights(nc):
    """Drop InstLdweights whose stationary operand is identical to the
    immediately preceding PE ldweights (the 4 matmuls of a unit share one
    stationary).  Deps/syncs of a dropped ldweights move to the following
    matmul.  Walrus's own --enable-ldw-opt rejects pre-split InstLdweights,
    so this is done at the BIR level."""
    for fn in nc.m.functions:
        for blk in fn.blocks:
            keep = []
            last_sig = None
            drop = []
            for ins in blk.instructions:
                tn = type(ins).__name__
                if getattr(ins, "engine", None) is not None and str(ins.engine) == "EngineType.PE":
                    if tn == "InstLdweights":
                        ap = ins.ins[0]
                        sig = (str(getattr(ap, "memref", None)), ap.offset,
                               str(ap.ap), str(getattr(ap, "dtype", None)))
                        if sig == last_sig and ins.sync_info is None:
                            drop.append(ins)
                            continue
                        last_sig = sig
                    elif tn != "InstMatmult":
                        last_sig = None
                keep.append(ins)
                if drop and tn == "InstMatmult":
                    for dins in drop:
                        ins.merge_dependencies_from(dins)
                    drop = []
            assert not drop
            blk.instructions[:] = keep


def _get_nc():
    if "nc" not in _CACHE:
        _CACHE["nc"] = _build()
    return _CACHE["nc"]


def kernel(stacked_batch: np.ndarray) -> np.ndarray:
    global LAST_RESULTS
    import ml_dtypes
    from concourse.bass_utils import run_bass_kernel_spmd

    nc = _get_nc()
    xf = np.ascontiguousarray(np.asarray(stacked_batch, dtype=np.float32))
    assert xf.shape == (N, D)

    ident = np.eye(128, dtype=np.float32)
    in_maps = [
        {"x": np.ascontiguousarray(np.roll(xf, -c * RPC, axis=0)), "ident": ident}
        for c in range(NCORES)
    ]
    res = run_bass_kernel_spmd(
        nc,
        in_maps,
        core_ids=list(range(NCORES)),
        trace=bool(os.environ.get("BASS_TRACE")),
    )
    LAST_RESULTS = res

    total = 0.0
    idx = np.arange(128)
    for c in range(NCORES):
        racc = np.asarray(res.results[c]["racc"], dtype=np.float64)  # [128, 64]
        etblk = np.asarray(res.results[c]["etblk"])                  # [128, 2048] i16
        vals = etblk.view(np.uint16).view(ml_dtypes.bfloat16).astype(np.float64)
        # rowsum for local row r = m*128 + p: sum ACT + DVE partials over g
        rowsum = np.zeros((128, M))
        for g in range(G):
            for m in range(M):
                u = g * M + m
                rowsum[:, m] += racc[:, u] + racc[:, G * M + u]
        rowsum[:, 0] += racc[:, 2 * G * M]  # split first-unit extra slot
        exp_diag = np.empty((128, M))
        exp_pos = np.empty((128, M))
        for m in range(M):
            exp_diag[:, m] = vals[idx, m * 128 + idx]
            exp_pos[:, m] = vals[idx, RPC + m * 128 + idx]
        loss = np.log(rowsum - exp_diag) - np.log(exp_pos)
        total += float(loss.sum())
    return np.float32(total / N)


# revision 46
# speedup vs baseline: 1.0315x; 1.0084x over previous
"""NT-Xent loss kernel for Trainium2, 8 NeuronCores.

Row-sharded similarity matrix with a multi-engine exponential pipeline:
  - Each core gets the full feature matrix cyclically rolled by c*1024 rows
    (identical SPMD program; its 1024 rows are always rolled-rows [0,1024)).
  - Group 0 preamble (latency-critical head): 512-col chunks on DVE
    (square, bf16 reduce, rnorm = exp(-0.5 ln) on ACT, broadcast scale),
    transposed via identity matmuls on the otherwise-idle PE into a bf16
    bitcast view of a PSUM ring slot; the first unit's matmuls are
    interleaved with the chunks and its ACT exp is split 512/1024 so the
    exp stream starts as early as possible.
  - Groups 1-3 preamble (throughput path): square + scale on GPSIMD
    (group 1's input DMA is held behind the head chain - GPSIMD and DVE
    contend for the shared SBUF port), bf16 reduce on DVE, and the
    transpose on the SBUF crossbar DMA (dma_start_transpose), which costs
    no engine time.
  - Main loop, 32 units of [128 rows x 2048 cols]: bf16 matmuls (fp8
    DoubleRow measured SLOWER than bf16 on this hw) into PSUM f32, one
    LDWEIGHTS per unit (redundant ones deduped at the BIR level since
    walrus --enable-ldw-opt rejects pre-split InstLdweights).  The exp is
    split between ACT (native Exp, fused f32 accum_out row sums, cols
    [0:1536]) and DVE (Schraudolph exp: one fused tensor_scalar
    f32->int16 whose rounded codes ARE the bf16 bit pattern of exp(x),
    mean-centered via the magic offset, plus a bf16 tensor_reduce for the
    row sums, cols [1536:2048]).  The last unit is ACT-only so the tail
    is just the accumulator read and the result DMA.
  - The diagonal (g=0) and positive-pair (g=2) [128,128] code blocks are
    collected by GPSIMD copies and shipped raw; the host extracts the
    diagonals, computes ln(rowsum - exp_diag) - ln(exp_pos) in f64, and
    means across cores - there is no device epilogue.
"""

import os

import numpy as np

N = 8192
D = 128
NCORES = 8
RPC = N // NCORES          # rows per core = 1024
G = 4                      # column groups
GCOLS = N // G             # 2048 columns per group
M = RPC // 128             # row tiles per core = 8
ACOLS = 1536               # ACT's exp share per 2048-col tile (mult of 128)
ZSCALE = 1.0               # z kept at unit scale (bf16 matmul)
ESC = 10.0                 # exp scale on raw psum values
SCH_A = 1846.649652        # ESC * 2^7 * log2(e)   (Schraudolph slope)
SCH_B = 16248.7807255      # 127*2^7 mean-centered (Schraudolph offset)

_CACHE = {}
LAST_RESULTS = None


def _patch_act_tables():
    """Force Exp/Ln onto the combined natural_log_exp_and_others table set
    so a single ACT table load covers the whole kernel."""
    if _CACHE.get("act_patched"):
        return
    import functools

    import concourse.bacc as bacc_mod
    import concourse.bass_interp as interp_mod
    import concourse.hw_specs as hw_specs
    import concourse.mybir as mybir

    AF = mybir.ActivationFunctionType
    orig = hw_specs.get_activation_tables

    @functools.cache
    def patched(arch):
        out = {}
        for name, funcs in orig(arch).items():
            if name != "natural_log_exp_and_others":
                funcs = funcs - {AF.Exp, AF.Ln}
            out[name] = funcs
        return out

    hw_specs.get_activation_tables = patched
    bacc_mod.get_activation_tables = patched
    interp_mod.get_activation_tables = patched
    _CACHE["act_patched"] = True


def _patch_ldw_opt():
    """Let walrus dedup consecutive identical LDWEIGHTS; the DoubleRow
    stationary is reused by 4 consecutive matmuls per unit."""
    if _CACHE.get("ldw_patched"):
        return
    import concourse.bass_utils as bu

    orig = bu.run_command

    def run2(argv, **kw):
        argv = [
            "--enable-ldw-opt=true" if a == "--enable-ldw-opt=false" else a
            for a in argv
        ]
        return orig(argv, **kw)

    bu.run_command = run2
    _CACHE["ldw_patched"] = True


def _build():
    import concourse.mybir as mybir
    import concourse.tile as tile
    from concourse import bacc

    _patch_act_tables()

    from bass_rust import add_dep_helper

    f32 = mybir.dt.float32
    bf16 = mybir.dt.bfloat16
    i16 = mybir.dt.int16
    AX = mybir.AxisListType
    OP = mybir.AluOpType
    AF = mybir.ActivationFunctionType

    nc = bacc.Bacc(
        "TRN2",
        target_bir_lowering=False,
        debug=False,
        enable_asserts=False,
        num_devices=NCORES,
    )
    x = nc.dram_tensor("x", [N, D], f32, kind="ExternalInput").ap()
    ident_in = nc.dram_tensor("ident", [128, 128], f32, kind="ExternalInput").ap()
    racc_out = nc.dram_tensor("racc", [128, 2 * G * M + 1], f32, kind="ExternalOutput").ap()
    etblk_out = nc.dram_tensor("etblk", [128, 2 * RPC], i16, kind="ExternalOutput").ap()

    with tile.TileContext(nc) as tc:
        with (
            tc.tile_pool(name="const", bufs=1) as constp,
            tc.tile_pool(name="xin", bufs=3) as xinp,
            tc.tile_pool(name="sq", bufs=2) as sqp,
            tc.tile_pool(name="small", bufs=2) as smallp,
            tc.tile_pool(name="zb", bufs=2) as zbp,
            tc.tile_pool(name="zbT", bufs=2) as zbTp,
            tc.tile_pool(name="zbT0", bufs=1) as zbT0p,
            tc.tile_pool(name="et", bufs=12) as etp,
            tc.tile_pool(name="acc", bufs=1) as accp,
            tc.tile_pool(name="psum", bufs=2, space="PSUM") as psump,
        ):
            # constants
            eps2 = constp.tile([128, 1], f32, tag="eps2")
            nc.vector.memset(eps2[:], 1e-16)
            identf = constp.tile([128, 128], f32, tag="identf")
            nc.sync.dma_start(out=identf[:], in_=ident_in)
            identb = constp.tile([128, 128], bf16, tag="identb")
            nc.vector.tensor_copy(identb[:], identf[:])

            racc = accp.tile([128, 2 * G * M + 1], f32, tag="racc")
            etcol = accp.tile([128, 2 * RPC], i16, tag="etcol")

            xgs = [None] * G
            zbTs = [None] * G
            ets = [None] * M

            def load_part(g, two_rings=False, after=None):
                """DMA group g's 2048 rolled rows of x into SBUF.  after=
                holds the transfers behind an instruction (keeps GPSIMD's
                dependent square off the head's critical window)."""
                xg = xinp.tile([128, GCOLS], f32, tag="xg")
                for q in range(4):
                    src = x[g * GCOLS + q * 512 : g * GCOLS + (q + 1) * 512, :]
                    src = src.rearrange("(s p) d -> p s d", p=128)
                    dst = xg[:, q * 512 : (q + 1) * 512].rearrange(
                        "p (s d) -> p s d", s=4
                    )
                    e = nc.scalar if (two_rings and q % 2 == 0) else nc.sync
                    ins = e.dma_start(out=dst, in_=src)
                    if after is not None:
                        add_dep_helper(ins.ins, after.ins, sync=True,
                                       reason="delay load behind head")
                xgs[g] = xg

            prev_dve = [None]
            chain_ops = []

            def chain(ins, sync=False):
                # pin emission order of the preamble; sync=True enforces
                # across engines (GPSIMD work must not overlap the head's
                # DVE chain - they contend for the shared SBUF port)
                if prev_dve[0] is not None:
                    add_dep_helper(ins.ins, prev_dve[0].ins, sync=sync,
                                   reason="preamble order")
                prev_dve[0] = ins
                chain_ops.append(ins)

            rnos = [None] * G
            sqs = [None] * G
            nsqs = [None] * G
            zbs = [None] * G

            def stage1_sq(g, sync=False):
                """Squares for group g's row norms on GPSIMD.  sync=True
                holds it until the head's DVE chain is done (shared SBUF
                port contention)."""
                xg = xgs[g]
                sq = sqp.tile([128, GCOLS], bf16, tag="sq", name="sq")
                nc.gpsimd.tensor_tensor(sq[:], xg[:], xg[:], op=OP.mult)
                sqs[g] = sq

            def stage1_red(g):
                """Reduce to ||x||^2 (DVE), then rno = exp(-0.5 ln) on ACT."""
                sq = sqs[g]
                nsq = smallp.tile([128, 16], bf16, tag="nsq")
                with nc.allow_low_precision(reason="bf16 norm, 2e-2 tol"):
                    chain(nc.vector.tensor_reduce(
                        nsq[:],
                        sq[:].rearrange("p (s d) -> p s d", s=16),
                        axis=AX.X,
                        op=OP.add,
                    ))
                lnv = smallp.tile([128, 16], f32, tag="lnv")
                nc.scalar.activation(lnv[:], nsq[:], AF.Ln, bias=eps2[:, 0:1])
                rno = smallp.tile([128, 16], f32, tag="rno")
                nc.scalar.activation(rno[:], lnv[:], AF.Exp, scale=-0.5)
                rnos[g] = rno

            def stage2(g):
                """Scale to z (bf16, GPSIMD) and crossbar-transpose, in two
                1024-col halves with separate zb tiles so the first half's
                transpose fires as soon as its scale is done (DMA deps are
                tile-granular)."""
                xg, rno = xgs[g], rnos[g]
                zbT = (zbT0p if g == 0 else zbTp).tile(
                    [128, GCOLS], bf16, tag="zbT", name="zbT")
                for h in range(2):
                    hs = slice(h * 1024, (h + 1) * 1024)
                    ss = slice(h * 8, (h + 1) * 8)
                    zb = zbp.tile([128, 1024], bf16, tag=f"zb{h}",
                                  name=f"zb{h}")
                    nc.gpsimd.tensor_tensor(
                        zb[:].rearrange("p (s d) -> p s d", s=8),
                        xg[:, hs].rearrange("p (s d) -> p s d", s=8),
                        rno[:, ss].unsqueeze(-1).broadcast_to([128, 8, 128]),
                        op=OP.mult,
                    )
                    # crossbar transpose: zbT[d, s*128+r] = zb[r, s*128+d]
                    nc.sync.dma_start_transpose(
                        out=zbT[:, hs].rearrange("p (s r) -> p s r", s=8),
                        in_=zb[:],
                    )
                zbTs[g] = zbT

            def prep_head(g):
                """Group 0: chunked norms + scale on DVE, transposed on the
                (otherwise idle) PE via identity matmuls - much lower
                latency than the crossbar DMA, so the first matmuls start
                ~6us earlier."""
                xg = xgs[g]
                sq = sqp.tile([128, GCOLS], bf16, tag="sq")
                nsq = smallp.tile([128, 16], bf16, tag="nsq")
                lnv = smallp.tile([128, 16], f32, tag="lnv")
                rno = smallp.tile([128, 16], f32, tag="rno")
                zbc = [zbp.tile([128, 512], bf16, tag=f"zbh{c}",
                                name=f"zbh{c}") for c in range(4)]
                zbT = zbT0p.tile([128, GCOLS], bf16, tag="zbT")
                # transpose bounce: a bf16 view of one main psum ring slot
                # (unit(0,0) takes the other slot, so no extra banks and no
                # wait on the first matmuls)
                pth = psump.tile([128, GCOLS], f32, tag="pt", name="pth")
                pthb = pth[:].bitcast(bf16)

                def norm_c(c):
                    cs = slice(c * 512, (c + 1) * 512)
                    ss = slice(c * 4, (c + 1) * 4)
                    chain(nc.vector.tensor_tensor(sq[:, cs], xg[:, cs],
                                                  xg[:, cs], op=OP.mult))
                    with nc.allow_low_precision(reason="bf16 norm, 2e-2 tol"):
                        chain(nc.vector.tensor_reduce(
                            nsq[:, ss],
                            sq[:, cs].rearrange("p (s d) -> p s d", s=4),
                            axis=AX.X,
                            op=OP.add,
                        ))
                    nc.scalar.activation(lnv[:, ss], nsq[:, ss], AF.Ln,
                                         bias=eps2[:, 0:1])
                    nc.scalar.activation(rno[:, ss], lnv[:, ss], AF.Exp,
                                         scale=-0.5)

                def scale_c(c):
                    cs = slice(c * 512, (c + 1) * 512)
                    ss = slice(c * 4, (c + 1) * 4)
                    zb = zbc[c]
                    chain(nc.vector.tensor_tensor(
                        zb[:].rearrange("p (s d) -> p s d", s=4),
                        xg[:, cs].rearrange("p (s d) -> p s d", s=4),
                        rno[:, ss].unsqueeze(-1).broadcast_to([128, 4, 128]),
                        op=OP.mult,
                    ))
                    for j in range(4):
                        nc.tensor.transpose(
                            pthb[:, c * 1024 + j * 128 : c * 1024 + (j + 1) * 128],
                            zb[:, j * 128 : (j + 1) * 128],
                            identb[:],
                        )
                    chain(nc.vector.tensor_copy(
                        zbT[:, cs], pthb[:, c * 1024 : c * 1024 + 512]
                    ))

                zbTs[g] = zbT
                pt0 = psump.tile([128, GCOLS], f32, tag="pt", name="pt0")
                norm_c(0)
                for c in range(1, 4):
                    norm_c(c)
                    scale_c(c - 1)
                    unit_mm(0, 0, pt0, c - 1)
                scale_c(3)
                unit_mm(0, 0, pt0, 3)
                return pt0

            def unit_mm(g, m, pt, k):
                lhs = zbTs[0][:, m * 128 : (m + 1) * 128]
                nc.tensor.matmul(
                    pt[:, k * 512 : (k + 1) * 512],
                    lhs,
                    zbTs[g][:, k * 512 : (k + 1) * 512],
                )

            def unit(g, m, pt=None):
                """One [128 rows x 2048 cols] tile: matmuls + split exp."""
                u = g * M + m
                if pt is None:
                    pt = psump.tile([128, GCOLS], f32, tag="pt")
                    for k in range(4):
                        unit_mm(g, m, pt, k)
                et = etp.tile([128, GCOLS], i16, tag="et")
                # ACT: true exp, fused row-sum accumulation.  The very
                # first unit is split so the exp stream starts right after
                # the first 512-col transpose chunk.
                if u == 0:
                    nc.scalar.activation(
                        et[:, 0:512].bitcast(bf16),
                        pt[:, 0:512],
                        AF.Exp,
                        scale=ESC,
                        accum_out=racc[:, 2 * G * M : 2 * G * M + 1],
                    )
                    nc.scalar.activation(
                        et[:, 512:ACOLS].bitcast(bf16),
                        pt[:, 512:ACOLS],
                        AF.Exp,
                        scale=ESC,
                        accum_out=racc[:, u : u + 1],
                    )
                else:
                    nc.scalar.activation(
                        et[:, 0:ACOLS].bitcast(bf16),
                        pt[:, 0:ACOLS],
                        AF.Exp,
                        scale=ESC,
                        accum_out=racc[:, u : u + 1],
                    )
                if u == G * M - 1:
                    # last unit: ACT takes the whole tile so the kernel tail
                    # is just the accumulator read + the racc DMA
                    nc.scalar.activation(
                        et[:, ACOLS:GCOLS].bitcast(bf16),
                        pt[:, ACOLS:GCOLS],
                        AF.Exp,
                        scale=ESC,
                        accum_out=racc[:, G * M + u : G * M + u + 1],
                    )
                else:
                    # DVE: Schraudolph exp (int16 codes = bf16 bit pattern)
                    nc.vector.tensor_scalar(
                        et[:, ACOLS:GCOLS], pt[:, ACOLS:GCOLS],
                        SCH_A, SCH_B, OP.mult, OP.add,
                    )
                    nc.vector.tensor_reduce(
                        racc[:, G * M + u : G * M + u + 1],
                        et[:, ACOLS:GCOLS].bitcast(bf16),
                        axis=AX.X,
                        op=OP.add,
                    )
                if g == 0 or g == 2:
                    ets[m] = (g, et)

            def collect_etblk():
                """Collect the held diag/pos code blocks on GPSIMD (emitted
                after the next group's stage2 so the GPS preamble never
                blocks) and ship the half to HBM."""
                half = None
                for mm in range(M):
                    gsrc, et = ets[mm]
                    half = 0 if gsrc == 0 else 1
                    nc.gpsimd.tensor_copy(
                        etcol[:, half * RPC + mm * 128 : half * RPC + (mm + 1) * 128],
                        et[:, mm * 128 : (mm + 1) * 128],
                    )
                    ets[mm] = None
                nc.sync.dma_start(
                    out=etblk_out[:, half * RPC : (half + 1) * RPC],
                    in_=etcol[:, half * RPC : (half + 1) * RPC],
                )

            # ---- pipeline ----
            load_part(0, two_rings=True)
            # warm the ACT Ln/Exp table while the input DMAs run
            warm = constp.tile([128, 1], f32, tag="warm")
            nc.vector.memset(warm[:], 1.0)
            nc.scalar.activation(warm[:], warm[:], AF.Ln)
            nc.scalar.activation(warm[:], warm[:], AF.Exp)
            pt0 = prep_head(0)
            load_part(1, after=chain_ops[10])
            stage1_sq(1)
            stage1_red(1)
            unit(0, 0, pt=pt0)
            for g in range(G):
                if g + 2 < G:
                    load_part(g + 2)
                for m in range(M):
                    if not (g == 0 and m == 0):
                        unit(g, m)
                    if g + 1 < G and m == 0:
                        stage2(g + 1)
                    if m == 2 and g in (1, 3) and ets[0] is not None:
                        collect_etblk()
                    if g + 2 < G and m == 4:
                        stage1_sq(g + 2)
                    if g + 2 < G and m == 6:
                        stage1_red(g + 2)
            

            nc.sync.dma_start(out=racc_out, in_=racc[:])

    _dedup_ldweights(nc)
    nc.compile()
    return nc


def _dedup_ldweights(nc):
    """Drop InstLdweights whose stationary operand is identical to the
    immediately preceding PE ldweights (the 4 matmuls of a unit share one
    stationary).  Deps/syncs of a dropped ldweights move to the following
    matmul.  Walrus's own --enable-ldw-opt rejects pre-split InstLdweights,
    so this is done at the BIR level."""
    for fn in nc.m.functions:
        for blk in fn.blocks:
            keep = []
            last_sig = None
            drop = []
            for ins in blk.instructions:
                tn = type(ins).__name__
                if getattr(ins, "engine", None) is not None and str(ins.engine) == "EngineType.PE":
                    if tn == "InstLdweights":
                        ap = ins.ins[0]
                        sig = (str(getattr(ap, "memref", None)), ap.offset,
                               str(ap.ap), str(getattr(ap, "dtype", None)))
                        if sig == last_sig and ins.sync_info is None:
                            drop.append(ins)
                            continue
                        last_sig = sig
                    elif tn != "InstMatmult":
                        last_sig = None
                keep.append(ins)
                if drop and tn == "InstMatmult":
                    for dins in drop:
                        ins.merge_dependencies_from(dins)
                    drop = []
            assert not drop
            blk.instructions[:] = keep


def _get_nc():
    if "nc" not in _CACHE:
        _CACHE["nc"] = _build()
    return _CACHE["nc"]


def kernel(stacked_batch: np.ndarray) -> np.ndarray:
    global LAST_RESULTS
    import ml_dtypes
    from concourse.bass_utils import run_bass_kernel_spmd

    nc = _get_nc()
    xf = np.ascontiguousarray(np.asarray(stacked_batch, dtype=np.float32))
    assert xf.shape == (N, D)

    ident = np.eye(128, dtype=np.float32)
    in_maps = [
        {"x": np.ascontiguousarray(np.roll(xf, -c * RPC, axis=0)), "ident": ident}
        for c in range(NCORES)
    ]
    res = run_bass_kernel_spmd(
        nc,
        in_maps,
        core_ids=list(range(NCORES)),
        trace=bool(os.environ.get("BASS_TRACE")),
    )
    LAST_RESULTS = res

    total = 0.0
    idx = np.arange(128)
    for c in range(NCORES):
        racc = np.asarray(res.results[c]["racc"], dtype=np.float64)  # [128, 64]
        etblk = np.asarray(res.results[c]["etblk"])                  # [128, 2048] i16
        vals = etblk.view(np.uint16).view(ml_dtypes.bfloat16).astype(np.float64)
        # rowsum for local row r = m*128 + p: sum ACT + DVE partials over g
        rowsum = np.zeros((128, M))
        for g in range(G):
            for m in range(M):
                u = g * M + m
                rowsum[:, m] += racc[:, u] + racc[:, G * M + u]
        rowsum[:, 0] += racc[:, 2 * G * M]  # split first-unit extra slot
        exp_diag = np.empty((128, M))
        exp_pos = np.empty((128, M))
        for m in range(M):
            exp_diag[:, m] = vals[idx, m * 128 + idx]
            exp_pos[:, m] = vals[idx, RPC + m * 128 + idx]
        loss = np.log(rowsum - exp_diag) - np.log(exp_pos)
        total += float(loss.sum())
    return np.float32(total / N)
